# revision 23
# baseline (speedup 1.0000x reference)
"""Trainium2 Bass kernel for CLIP-style symmetric contrastive loss.

Problem: image_features [8192, 1024] f32, text_features [8192, 1024] f32.
  loss = 0.5 * (CE(logits, diag) + CE(logits.T, diag)),
  logits = cosine_similarity(img, txt) / 0.07.

Distribution: shard image rows across 8 NeuronCores. Each core computes its
[8192, 1024] slab of logits TRANSPOSED -- text columns on PSUM partitions,
image rows on the free axis -- via fp8 DoubleRow matmuls (2x PE rate) with
raw (unnormalized) text as the stationary operand. The per-text-column
C/||t_j|| rides the ACT exp's per-partition *scale* operand.

vs the 281us baseline:
  - Text-column norms: each core computes ssq only for its OWN 1024-row text
    slab (from the bf16 totT copy it already holds for the diagonal dots),
    then one early AllGather distributes all 8192 ssq values. rsqrt runs as
    a Newton iteration (Quake seed) on the Vector engine, so the Scalar
    engine never loads the Sqrt table mid-loop.
  - Chunks 0-4 additionally compute their norms locally (fp8 squares on
    DVE/GpSimd + DR ones-matmuls) to cover the AllGather's latency.
  - Main loop is pure: 32 DR matmuls -> 4 exps -> 4 rowsum DR matmuls per
    chunk. No mid-loop norm prep, no PSUM contention, no ACT table swaps.
  - Tail: one AllGather of a [128, 68] bf16 payload (colsums bf16 +
    R/Draw f32 bitcast into bf16 pairs) in p-major layout, then 7 vector
    adds. Replaces the 30us AllReduce + strided-scatter tail.

Math (C = 1/T bounds every logit; LOG_OFF keeps exp outputs ~O(1) for fp8):
  loss = (C - LOG_OFF) + (R + L - 2C * Draw) / (2N)
    R    = sum_i log sum_j exp(C*s_ij - C + LOG_OFF)
    L    = same over columns
    Draw = sum_i cos(img_i, txt_i)
"""
import math
import threading
from contextlib import ExitStack

import ml_dtypes
import numpy as np

import concourse.bacc as bacc
import concourse.bass as bass
import concourse.bass_isa as bass_isa
import concourse.mybir as mybir
import concourse.tile as tile
from concourse.bass_utils import run_bass_kernel_spmd

F32 = mybir.dt.float32
BF16 = mybir.dt.bfloat16
FP8 = mybir.dt.float8e4
U32 = mybir.dt.uint32
I32 = mybir.dt.int32
AF = mybir.ActivationFunctionType
ALU = mybir.AluOpType
DR = mybir.MatmulPerfMode.DoubleRow

N_CORES = 8
N = 8192
D = 1024
TEMPERATURE = 0.07
MAGIC = 0x5F3759DF
N_LOCAL = 5          # chunks whose text norms are computed locally per-core


def build_nc(n=N, d=D, n_cores=N_CORES):
    nc = bacc.Bacc("TRN2", target_bir_lowering=False, debug=False,
                   num_devices=n_cores)
    rows = n // n_cores
    P = 128
    kt = d // P
    CH = 512
    imgT = nc.dram_tensor("imgT", [rows // CH, P, kt, CH], BF16,
                          kind="ExternalInput").ap()
    totT = nc.dram_tensor("totT", [rows // CH, P, kt, CH], BF16,
                          kind="ExternalInput").ap()
    txt8 = nc.dram_tensor("txt8", [n // CH, P, kt, CH], FP8,
                          kind="ExternalInput").ap()
    ones8 = nc.dram_tensor("ones8", [128, 256], FP8, kind="ExternalInput").ap()
    onesb = nc.dram_tensor("onesb", [128, 128], BF16, kind="ExternalInput").ap()
    out = nc.dram_tensor("out", [1, 1], F32, kind="ExternalOutput").ap()

    with tile.TileContext(nc) as tc:
        _body(tc, imgT, totT, txt8, ones8, onesb, out,
              n=n, d=d, rows=rows, n_cores=n_cores)
    nc.compile()
    return nc


def _newton_rsqrt(nc, y, a, t1, scale=1.0):
    """y = scale * rsqrt(a), all [P, X] f32 SBUF tiles (a preserved).

    Quake-III seed on the Vector engine (int32 add; uint32 NaNs out) + two
    Newton steps: rel err ~5e-6. Keeps the Scalar engine's activation table
    free of Sqrt so Exp/Ln never reload mid-loop.
    """
    nc.vector.tensor_scalar(y.bitcast(U32), a.bitcast(U32), 1, 0xFFFFFFFF,
                            ALU.logical_shift_right, ALU.bitwise_xor)
    nc.vector.tensor_scalar(y.bitcast(I32), y.bitcast(I32),
                            MAGIC + 1, None, ALU.add)
    for it in range(2):
        nc.vector.tensor_tensor(t1, y, y, ALU.mult)
        nc.vector.tensor_tensor(t1, t1, a, ALU.mult)
        nc.vector.tensor_scalar(t1, t1, -0.5, 1.5, ALU.mult, ALU.add)
        if it == 1 and scale != 1.0:
            nc.vector.tensor_scalar(y, y, float(scale), None, ALU.mult)
        nc.vector.tensor_tensor(y, y, t1, ALU.mult)


def _body(tc, imgT, totT, txt8, ones8, onesb, out, *, n, d, rows, n_cores):
    nc = tc.nc
    P = 128
    kt = d // P                      # 8 contraction tiles
    kp = kt // 2                     # 4 DoubleRow k-pairs
    CH = 512
    n_ch = n // CH                   # 16 text column chunks
    jb_n = CH // P                   # 4 j-blocks per chunk
    rc_ch = rows // CH               # 2 image row chunks
    inv_t = float(1.0 / TEMPERATURE)
    log_off = float(20.0 * math.log(2.0))
    ebias_v = float(log_off - inv_t)         # exp bias: -C + LOG_OFF
    WP = 68                                  # tail payload cols (64+2*2)

    with ExitStack() as ctx:
        persist = ctx.enter_context(tc.tile_pool(name="persist", bufs=1))
        sqbp = ctx.enter_context(tc.tile_pool(name="sqbp", bufs=2))
        sq8p = ctx.enter_context(tc.tile_pool(name="sq8p", bufs=2))
        rsp = ctx.enter_context(tc.tile_pool(name="rsp", bufs=2))
        exsp = ctx.enter_context(tc.tile_pool(name="exsp", bufs=2))
        v1 = ctx.enter_context(tc.tile_pool(name="v1", bufs=1))
        psum_mm = ctx.enter_context(tc.tile_pool(name="psum_mm", bufs=2,
                                                 space="PSUM"))
        psum_n = ctx.enter_context(tc.tile_pool(name="psum_n", bufs=1,
                                                space="PSUM"))
        dram = ctx.enter_context(tc.tile_pool(name="dram", bufs=1,
                                              space="DRAM"))

        txt8s = persist.tile([P, n_ch, kt, CH], FP8, tag="txt8s")   # 8 MB
        img8 = persist.tile([P, rc_ch, kt, CH], FP8, tag="img8")    # 1 MB
        imgT_sb = persist.tile([P, rc_ch, kt, CH], BF16, tag="imgT")
        totT_sb = persist.tile([P, rc_ch, kt, CH], BF16, tag="totT")
        rs_i = persist.tile([P, rows], F32, tag="rs_i")
        ddv = persist.tile([P, rows], F32, tag="ddv")
        tssq = persist.tile([P, rc_ch, CH], F32, tag="tssq")   # own-text ssq
        rs_to = persist.tile([P, rows], F32, tag="rs_to")
        scl_loc = persist.tile([P, N_LOCAL, jb_n], F32, tag="scl_loc")
        scl_gat = persist.tile([P, n_ch * jb_n], F32, tag="scl_gat")
        pay = persist.tile([P, WP], F32, tag="pay")   # 0-63 cparts, 64 R, 65 D
        pay_bf = persist.tile([P, WP], BF16, tag="pay_bf")
        recv = persist.tile([P, n_cores, WP], BF16, tag="recv")
        ones8_sb = persist.tile([P, 2, P], FP8, tag="ones8")
        onesb_sb = persist.tile([P, P], BF16, tag="onesb")
        ebias = persist.tile([P, 1], F32, tag="ebias")
        vecs = persist.tile([P, 16], F32, tag="vecs")
        rps = psum_n.tile([P, rows], F32, tag="rps")  # rowsum accum, 2 banks

        agin = dram.tile([1, rows], F32, tag="agin")
        agout = dram.tile([1, n], F32, tag="agout", addr_space="Shared")
        bnc = dram.tile([1, N_LOCAL, CH], F32, tag="bnc")
        cbuf = dram.tile([1, P * WP], BF16, tag="cbuf")
        cbuf_out = dram.tile([1, n_cores * P * WP], BF16, tag="cbuf_out",
                             addr_space="Shared")
        grp = [list(range(n_cores))]

        # --- constants + input DMAs ----------------------------------------
        nc.sync.dma_start(ones8_sb[:, 0, :], ones8[:, 0:P])
        nc.sync.dma_start(ones8_sb[:, 1, :], ones8[:, P:2 * P])
        nc.sync.dma_start(onesb_sb[:], onesb[:])
        nc.gpsimd.memset(ebias[:], ebias_v)

        # ALL input DMAs on the sync queue: issuing the 16 txt chunks from
        # the Scalar queue was serializing the ACT engine behind ~20us of
        # DMA-issue overhead, pushing the Sqrt table load (and hence the
        # first exp) to ~42us/~77us.
        for rc in range(rc_ch):
            nc.sync.dma_start(imgT_sb[:, rc, :, :], imgT[rc])
        for rc in range(rc_ch):
            nc.sync.dma_start(totT_sb[:, rc, :, :], totT[rc])
        for c in range(n_ch):
            nc.sync.dma_start(txt8s[:, c, :, :], txt8[c])

        def col_ssq_bf16(src_ap, ps):
            """colsum(src^2) for a bf16 [P, kt, CH] slab -> f32 PSUM [P, CH].

            Squares quantize to fp8 so the ones-matmuls run DoubleRow:
            4 matmuls at 0.5 cyc/row instead of 8 bf16 at 1 cyc/row.
            """
            sq = sq8p.tile([P, kt, CH], FP8, tag="sq8")
            nc.vector.tensor_tensor(sq[:], src_ap, src_ap, ALU.mult)
            for t in range(kp):
                nc.tensor.matmul(ps[:], ones8_sb[:],
                                 sq[:, 2 * t:2 * t + 2, :],
                                 start=(t == 0), stop=(t == kp - 1),
                                 perf_mode=DR)

        # --- image norms + fp8 quantize ------------------------------------
        for rc in range(rc_ch):
            sl = slice(rc * CH, (rc + 1) * CH)
            ps = psum_n.tile([P, CH], F32, tag="nps", bufs=2)
            col_ssq_bf16(imgT_sb[:, rc, :, :], ps)
            r32 = rsp.tile([P, CH], F32, tag="r32")
            nc.vector.reciprocal_approx_fast(r32[:], ps[:])
            # Sqrt ACTs all precede the first Exp -> one table swap total
            nc.scalar.activation(rs_i[:, sl], r32[:], AF.Sqrt)
            for k in range(kt):
                nc.vector.tensor_tensor(img8[:, rc, k, :],
                                        imgT_sb[:, rc, k, :],
                                        rs_i[:, sl], ALU.mult)

        # --- local text norms for the first N_LOCAL chunks ------------------
        # (cover the norms-AllGather latency; squares for c0/c1 on GpSimd,
        # the rest on DVE after the image quantize)
        def prep_local(c, eng):
            sq = sq8p.tile([P, kt, CH], FP8, tag="sq8")
            eng.tensor_tensor(sq[:], txt8s[:, c, :, :], txt8s[:, c, :, :],
                              ALU.mult)
            ps = psum_n.tile([P, CH], F32, tag="nps", bufs=2)
            for t in range(kp):
                nc.tensor.matmul(ps[:], ones8_sb[:],
                                 sq[:, 2 * t:2 * t + 2, :],
                                 start=(t == 0), stop=(t == kp - 1),
                                 perf_mode=DR)
            row = rsp.tile([1, CH], F32, tag="row", bufs=8)
            nc.vector.tensor_copy(row[:], ps[0:1, :])
            # partition-transpose [1, 512] -> [128, 4] via a DRAM bounce
            nc.sync.dma_start(bnc[0:1, c, :], row[:])
            aT = rsp.tile([P, jb_n], F32, tag="aT", bufs=8)
            nc.sync.dma_start(
                aT[:], bnc[0:1, c, :].rearrange("a (x p) -> (a p) x", p=P))
            t1 = rsp.tile([P, jb_n], F32, tag="t1l", bufs=8)
            _newton_rsqrt(nc, scl_loc[:, c, :], aT[:], t1[:], scale=inv_t)

        prep_local(0, nc.gpsimd)
        prep_local(1, nc.gpsimd)

        # --- own-slab text ssq -> AllGather -> all 8192 text norms ----------
        for rc in range(rc_ch):
            ps2 = psum_n.tile([P, CH], F32, tag="nps", bufs=2)
            col_ssq_bf16(totT_sb[:, rc, :, :], ps2)
            nc.vector.tensor_copy(tssq[:, rc, :], ps2[:])
        # p-major transpose into DRAM: agin[(p x)] <- tssq rows [(x p)]
        arow = v1.tile([1, rows], F32, tag="arow")
        nc.vector.tensor_copy(arow[0:1, 0:CH], tssq[0:1, 0, :])
        nc.vector.tensor_copy(arow[0:1, CH:rows], tssq[0:1, 1, :])
        nc.sync.dma_start(
            agin[0:1, :].rearrange("a (p x) -> (a x) p", p=P),
            arow[0:1, :])
        nc.gpsimd.collective_compute(
            "AllGather", ALU.bypass, replica_groups=grp,
            ins=[agin[:].opt()], outs=[agout[:].opt()])
        # gather slots are rank-ordered == natural global column order
        gat = v1.tile([P, n_ch * jb_n], F32, tag="gat")
        nc.sync.dma_start(
            gat[:].rearrange("p (m x) -> p m x", x=rows // P),
            agout[0:1, :].rearrange("a (m p x) -> (a p) m x",
                                    p=P, x=rows // P))
        gt1 = v1.tile([P, n_ch * jb_n], F32, tag="gt1")
        _newton_rsqrt(nc, scl_gat[:], gat[:], gt1[:], scale=inv_t)

        # remaining local-chunk norms on DVE (after quantize in issue order)
        for c in range(2, N_LOCAL):
            prep_local(c, nc.vector)

        # --- diagonal dots (for Draw) --------------------------------------
        for rc in range(rc_ch):
            sl = slice(rc * CH, (rc + 1) * CH)
            prod = sqbp.tile([P, kt, CH], BF16, tag="sqb")
            nc.vector.tensor_tensor(prod[:], imgT_sb[:, rc, :, :],
                                    totT_sb[:, rc, :, :], ALU.mult)
            dps = psum_n.tile([P, CH], F32, tag="nps", bufs=2)
            for k in range(kt):
                nc.tensor.matmul(dps[:], onesb_sb[:], prod[:, k, :],
                                 start=(k == 0), stop=(k == kt - 1))
            nc.vector.tensor_copy(ddv[:, sl], dps[:])
        # rs_to = rsqrt(own text ssq), [P, rows] replicated
        for rc in range(rc_ch):
            sl = slice(rc * CH, (rc + 1) * CH)
            tt1 = rsp.tile([P, CH], F32, tag="tt1")
            _newton_rsqrt(nc, rs_to[:, sl], tssq[:, rc, :], tt1[:])
        nc.vector.tensor_tensor(ddv[:], ddv[:], rs_i[:], ALU.mult)
        nc.vector.tensor_tensor(ddv[:], ddv[:], rs_to[:], ALU.mult)
        nc.vector.tensor_reduce(pay[:, 65:66], ddv[:],
                                axis=mybir.AxisListType.X, op=ALU.add)

        # --- main loop: per text chunk --------------------------------------
        for c in range(n_ch):
            exs = exsp.tile([P, jb_n, rows], FP8, tag="exs")
            for jb in range(jb_n):
                col = c * jb_n + jb
                mm = psum_mm.tile([P, rows], F32, tag="mm")
                for rc in range(rc_ch):
                    for t in range(kp):
                        nc.tensor.matmul(
                            mm[:, rc * CH:(rc + 1) * CH],
                            txt8s[:, c, 2 * t:2 * t + 2, jb * P:(jb + 1) * P],
                            img8[:, rc, 2 * t:2 * t + 2, :],
                            start=(t == 0), stop=(t == kp - 1), perf_mode=DR)
                scl = (scl_loc[:, c, jb:jb + 1] if c < N_LOCAL
                       else scl_gat[:, col:col + 1])
                nc.scalar.activation(
                    exs[:, jb, :], mm[:], AF.Exp, bias=ebias[:, 0:1],
                    scale=scl, accum_out=pay[:, col:col + 1])
            # rowsum partials: DoubleRow ones-matmuls into persistent PSUM
            for u in range(jb_n // 2):
                for h in range(rc_ch):
                    nc.tensor.matmul(
                        rps[:, h * CH:(h + 1) * CH], ones8_sb[:],
                        exs[:, 2 * u:2 * u + 2, h * CH:(h + 1) * CH],
                        start=(c == 0 and u == 0),
                        stop=(c == n_ch - 1 and u == jb_n // 2 - 1),
                        perf_mode=DR)

        # --- local scalars ---------------------------------------------------
        lnr = v1.tile([P, rows], F32, tag="lnr")
        nc.scalar.activation(lnr[:], rps[:], AF.Ln)   # same table as Exp
        nc.vector.tensor_reduce(pay[:, 64:65], lnr[:],
                                axis=mybir.AxisListType.X, op=ALU.add)

        # --- tail AllGather: [128, 68] bf16 payload, p-major ---------------
        # cols 0-63: cparts as bf16; cols 64-67: R/Draw f32 bitcast pairs
        nc.vector.tensor_copy(pay_bf[:, 0:64], pay[:, 0:64])
        nc.vector.tensor_copy(pay_bf[:, 64:68].bitcast(F32), pay[:, 64:66])
        nc.sync.dma_start(
            cbuf[0:1, :].rearrange("a (p x) -> (a p) x", p=P), pay_bf[:])
        nc.gpsimd.collective_compute(
            "AllGather", ALU.bypass, replica_groups=grp,
            ins=[cbuf[:].opt()], outs=[cbuf_out[:].opt()])
        nc.sync.dma_start(
            recv[:],
            cbuf_out[0:1, :].rearrange("a (m p x) -> (a p) m x",
                                       p=P, x=WP))
        # sum colsums in f32; sum the bitcast scalar pairs as f32
        acc = v1.tile([P, 64], F32, tag="acc")
        accs = v1.tile([P, 2], F32, tag="accs")
        nc.vector.tensor_tensor(acc[:], recv[:, 0, 0:64], recv[:, 1, 0:64],
                                ALU.add)
        nc.vector.tensor_tensor(accs[:], recv[:, 0, 64:68].bitcast(F32),
                                recv[:, 1, 64:68].bitcast(F32), ALU.add)
        for m in range(2, n_cores):
            nc.vector.tensor_tensor(acc[:], acc[:], recv[:, m, 0:64], ALU.add)
            nc.vector.tensor_tensor(accs[:], accs[:],
                                    recv[:, m, 64:68].bitcast(F32), ALU.add)
        ln_cs = v1.tile([P, 64], F32, tag="ln_cs")
        nc.scalar.activation(ln_cs[:], acc[:], AF.Ln)
        nc.vector.tensor_reduce(vecs[:, 3:4], ln_cs[:],
                                axis=mybir.AxisListType.X, op=ALU.add)
        nc.gpsimd.partition_all_reduce(vecs[:, 4:5], vecs[:, 3:4], channels=P,
                                       reduce_op=bass_isa.ReduceOp.add)

        # loss = (C - LOG_OFF) + (R + L - 2C*Draw) / (2N)
        fin = v1.tile([P, 8], F32, tag="fin")
        nc.vector.tensor_tensor(fin[0:1, 0:1], accs[0:1, 0:1], vecs[0:1, 4:5],
                                ALU.add)                        # R + L
        nc.vector.tensor_scalar_mul(fin[0:1, 1:2], accs[0:1, 1:2],
                                    float(-2.0 * inv_t))        # -2C*Draw
        nc.vector.tensor_tensor(fin[0:1, 2:3], fin[0:1, 0:1], fin[0:1, 1:2],
                                ALU.add)
        nc.scalar.activation(fin[0:1, 3:4], fin[0:1, 2:3], AF.Copy,
                             bias=float(inv_t - log_off),
                             scale=float(1.0 / (2 * n)))
        nc.sync.dma_start(out[0:1, 0:1], fin[0:1, 3:4])


def _permute(xT, ch):
    """[d, cols] -> [cols//ch, 128, d//128, ch] (SBUF tile layout, dense)."""
    d, cols = xT.shape
    return np.ascontiguousarray(
        xT.reshape(d // 128, 128, cols // ch, ch).transpose(2, 1, 0, 3))


def make_in_maps(image_features, text_features, n=N, d=D, n_cores=N_CORES):
    image_features = np.asarray(image_features, dtype=np.float32)
    text_features = np.asarray(text_features, dtype=np.float32)
    rows = n // n_cores
    txt8 = _permute(text_features.T.astype(ml_dtypes.float8_e4m3), 512)
    ones8 = np.ones((128, 256), dtype=ml_dtypes.float8_e4m3)
    onesb = np.ones((128, 128), dtype=ml_dtypes.bfloat16)
    maps = []
    for m in range(n_cores):
        sl = slice(m * rows, (m + 1) * rows)
        maps.append({
            "imgT": _permute(
                image_features[sl].T.astype(ml_dtypes.bfloat16), 512),
            "totT": _permute(
                text_features[sl].T.astype(ml_dtypes.bfloat16), 512),
            "txt8": txt8,
            "ones8": ones8,
            "onesb": onesb,
        })
    return maps


_CACHE = {}
_LOCK = threading.Lock()


def _get_nc():
    with _LOCK:
        if "nc" not in _CACHE:
            _CACHE["nc"] = build_nc()
        return _CACHE["nc"]


def kernel(image_features, text_features):
    image_features = np.asarray(image_features, dtype=np.float32)
    text_features = np.asarray(text_features, dtype=np.float32)
    assert image_features.shape == (N, D) and text_features.shape == (N, D)
    nc = _get_nc()
    in_maps = make_in_maps(image_features, text_features)
    res = run_bass_kernel_spmd(nc, in_maps, list(range(N_CORES)))
    val = np.float32(res.results[0]["out"][0, 0])
    return np.array(val, dtype=np.float32)


# revision 24
# speedup vs baseline: 1.0145x; 1.0145x over previous
"""Trainium2 Bass kernel for CLIP-style symmetric contrastive loss.

Problem: image_features [8192, 1024] f32, text_features [8192, 1024] f32.
  loss = 0.5 * (CE(logits, diag) + CE(logits.T, diag)),
  logits = cosine_similarity(img, txt) / 0.07.

Distribution: shard image rows across 8 NeuronCores. Each core computes its
[8192, 1024] slab of logits TRANSPOSED -- text columns on PSUM partitions,
image rows on the free axis -- via fp8 DoubleRow matmuls (2x PE rate) with
raw (unnormalized) text as the stationary operand. The per-text-column
C/||t_j|| rides the ACT exp's per-partition *scale* operand.

vs the 281us baseline:
  - Text-column norms: each core computes ssq only for its OWN 1024-row text
    slab (from the bf16 totT copy it already holds for the diagonal dots),
    then one early AllGather distributes all 8192 ssq values. rsqrt runs as
    a Newton iteration (Quake seed) on the Vector engine, so the Scalar
    engine never loads the Sqrt table mid-loop.
  - Chunks 0-4 additionally compute their norms locally (fp8 squares on
    DVE/GpSimd + DR ones-matmuls) to cover the AllGather's latency.
  - Main loop is pure: 32 DR matmuls -> 4 exps -> 4 rowsum DR matmuls per
    chunk. No mid-loop norm prep, no PSUM contention, no ACT table swaps.
  - Tail: one AllGather of a [128, 68] bf16 payload (colsums bf16 +
    R/Draw f32 bitcast into bf16 pairs) in p-major layout, then 7 vector
    adds. Replaces the 30us AllReduce + strided-scatter tail.

Math (C = 1/T bounds every logit; LOG_OFF keeps exp outputs ~O(1) for fp8):
  loss = (C - LOG_OFF) + (R + L - 2C * Draw) / (2N)
    R    = sum_i log sum_j exp(C*s_ij - C + LOG_OFF)
    L    = same over columns
    Draw = sum_i cos(img_i, txt_i)
"""
import math
import threading
from contextlib import ExitStack

import ml_dtypes
import numpy as np

import concourse.bacc as bacc
import concourse.bass as bass
import concourse.bass_isa as bass_isa
import concourse.mybir as mybir
import concourse.tile as tile
from concourse.bass_utils import run_bass_kernel_spmd

F32 = mybir.dt.float32
BF16 = mybir.dt.bfloat16
FP8 = mybir.dt.float8e4
U32 = mybir.dt.uint32
I32 = mybir.dt.int32
AF = mybir.ActivationFunctionType
ALU = mybir.AluOpType
DR = mybir.MatmulPerfMode.DoubleRow

N_CORES = 8
N = 8192
D = 1024
TEMPERATURE = 0.07
MAGIC = 0x5F3759DF
N_LOCAL = 5          # chunks whose text norms are computed locally per-core


def build_nc(n=N, d=D, n_cores=N_CORES):
    nc = bacc.Bacc("TRN2", target_bir_lowering=False, debug=False,
                   num_devices=n_cores)
    rows = n // n_cores
    P = 128
    kt = d // P
    CH = 512
    imgT = nc.dram_tensor("imgT", [rows // CH, P, kt, CH], BF16,
                          kind="ExternalInput").ap()
    totT = nc.dram_tensor("totT", [rows // CH, P, kt, CH], BF16,
                          kind="ExternalInput").ap()
    txt8 = nc.dram_tensor("txt8", [n // CH, P, kt, CH], FP8,
                          kind="ExternalInput").ap()
    ones8 = nc.dram_tensor("ones8", [128, 256], FP8, kind="ExternalInput").ap()
    onesb = nc.dram_tensor("onesb", [128, 128], BF16, kind="ExternalInput").ap()
    out = nc.dram_tensor("out", [1, 1], F32, kind="ExternalOutput").ap()

    with tile.TileContext(nc) as tc:
        _body(tc, imgT, totT, txt8, ones8, onesb, out,
              n=n, d=d, rows=rows, n_cores=n_cores)
    nc.compile()
    return nc


def _newton_rsqrt(nc, y, a, t1, scale=1.0):
    """y = scale * rsqrt(a), all [P, X] f32 SBUF tiles (a preserved).

    Quake-III seed on the Vector engine (int32 add; uint32 NaNs out) + two
    Newton steps: rel err ~5e-6. Keeps the Scalar engine's activation table
    free of Sqrt so Exp/Ln never reload mid-loop.
    """
    nc.vector.tensor_scalar(y.bitcast(U32), a.bitcast(U32), 1, 0xFFFFFFFF,
                            ALU.logical_shift_right, ALU.bitwise_xor)
    nc.vector.tensor_scalar(y.bitcast(I32), y.bitcast(I32),
                            MAGIC + 1, None, ALU.add)
    for it in range(2):
        nc.vector.tensor_tensor(t1, y, y, ALU.mult)
        nc.vector.tensor_tensor(t1, t1, a, ALU.mult)
        nc.vector.tensor_scalar(t1, t1, -0.5, 1.5, ALU.mult, ALU.add)
        if it == 1 and scale != 1.0:
            nc.vector.tensor_scalar(y, y, float(scale), None, ALU.mult)
        nc.vector.tensor_tensor(y, y, t1, ALU.mult)


def _body(tc, imgT, totT, txt8, ones8, onesb, out, *, n, d, rows, n_cores):
    nc = tc.nc
    P = 128
    kt = d // P                      # 8 contraction tiles
    kp = kt // 2                     # 4 DoubleRow k-pairs
    CH = 512
    n_ch = n // CH                   # 16 text column chunks
    jb_n = CH // P                   # 4 j-blocks per chunk
    rc_ch = rows // CH               # 2 image row chunks
    inv_t = float(1.0 / TEMPERATURE)
    log_off = float(20.0 * math.log(2.0))
    ebias_v = float(log_off - inv_t)         # exp bias: -C + LOG_OFF
    WP = 68                                  # tail payload cols (64+2*2)

    with ExitStack() as ctx:
        persist = ctx.enter_context(tc.tile_pool(name="persist", bufs=1))
        sqbp = ctx.enter_context(tc.tile_pool(name="sqbp", bufs=2))
        sq8p = ctx.enter_context(tc.tile_pool(name="sq8p", bufs=2))
        rsp = ctx.enter_context(tc.tile_pool(name="rsp", bufs=2))
        exsp = ctx.enter_context(tc.tile_pool(name="exsp", bufs=2))
        v1 = ctx.enter_context(tc.tile_pool(name="v1", bufs=1))
        psum_mm = ctx.enter_context(tc.tile_pool(name="psum_mm", bufs=2,
                                                 space="PSUM"))
        psum_n = ctx.enter_context(tc.tile_pool(name="psum_n", bufs=1,
                                                space="PSUM"))
        dram = ctx.enter_context(tc.tile_pool(name="dram", bufs=1,
                                              space="DRAM"))

        txt8s = persist.tile([P, n_ch, kt, CH], FP8, tag="txt8s")   # 8 MB
        img8 = persist.tile([P, rc_ch, kt, CH], FP8, tag="img8")    # 1 MB
        imgT_sb = persist.tile([P, rc_ch, kt, CH], BF16, tag="imgT")
        totT_sb = persist.tile([P, rc_ch, kt, CH], BF16, tag="totT")
        rs_i = persist.tile([P, rows], F32, tag="rs_i")
        ddv = persist.tile([P, rows], F32, tag="ddv")
        tssq = persist.tile([P, rc_ch, CH], F32, tag="tssq")   # own-text ssq
        rs_to = persist.tile([P, rows], F32, tag="rs_to")
        scl_loc = persist.tile([P, N_LOCAL, jb_n], F32, tag="scl_loc")
        scl_gat = persist.tile([P, n_ch * jb_n], F32, tag="scl_gat")
        pay = persist.tile([P, WP], F32, tag="pay")   # 0-63 cparts, 64 R, 65 D
        pay_bf = persist.tile([P, WP], BF16, tag="pay_bf")
        recv = persist.tile([P, n_cores, WP], BF16, tag="recv")
        ones8_sb = persist.tile([P, 2, P], FP8, tag="ones8")
        onesb_sb = persist.tile([P, P], BF16, tag="onesb")
        ebias = persist.tile([P, 1], F32, tag="ebias")
        vecs = persist.tile([P, 16], F32, tag="vecs")
        rps = psum_n.tile([P, rows], F32, tag="rps")  # rowsum accum, 2 banks

        agin = dram.tile([1, rows], F32, tag="agin")
        agout = dram.tile([1, n], F32, tag="agout", addr_space="Shared")
        bnc = dram.tile([1, N_LOCAL, CH], F32, tag="bnc")
        cbuf = dram.tile([1, P * WP], BF16, tag="cbuf")
        cbuf_out = dram.tile([1, n_cores * P * WP], BF16, tag="cbuf_out",
                             addr_space="Shared")
        grp = [list(range(n_cores))]

        # --- constants + input DMAs ----------------------------------------
        nc.sync.dma_start(ones8_sb[:, 0, :], ones8[:, 0:P])
        nc.sync.dma_start(ones8_sb[:, 1, :], ones8[:, P:2 * P])
        nc.sync.dma_start(onesb_sb[:], onesb[:])
        nc.gpsimd.memset(ebias[:], ebias_v)

        # sync queue: image first (gates quantize -> main loop), then totT.
        # txt chunks issue from the Scalar queue: this costs ~20us of ACT
        # issue overhead before the first exp, but moving them to sync (or
        # splitting) measured WORSE (277-295us vs 272) - the scheduler
        # rebalances adversarially. Keep the empirically best layout.
        for rc in range(rc_ch):
            nc.sync.dma_start(imgT_sb[:, rc, :, :], imgT[rc])
        for rc in range(rc_ch):
            nc.sync.dma_start(totT_sb[:, rc, :, :], totT[rc])
        for c in range(n_ch):
            nc.scalar.dma_start(txt8s[:, c, :, :], txt8[c])

        def col_ssq_bf16(src_ap, ps):
            """colsum(src^2) for a bf16 [P, kt, CH] slab -> f32 PSUM [P, CH].

            Squares quantize to fp8 so the ones-matmuls run DoubleRow:
            4 matmuls at 0.5 cyc/row instead of 8 bf16 at 1 cyc/row.
            """
            sq = sq8p.tile([P, kt, CH], FP8, tag="sq8")
            nc.vector.tensor_tensor(sq[:], src_ap, src_ap, ALU.mult)
            for t in range(kp):
                nc.tensor.matmul(ps[:], ones8_sb[:],
                                 sq[:, 2 * t:2 * t + 2, :],
                                 start=(t == 0), stop=(t == kp - 1),
                                 perf_mode=DR)

        # --- image norms + fp8 quantize ------------------------------------
        for rc in range(rc_ch):
            sl = slice(rc * CH, (rc + 1) * CH)
            ps = psum_n.tile([P, CH], F32, tag="nps", bufs=2)
            col_ssq_bf16(imgT_sb[:, rc, :, :], ps)
            r32 = rsp.tile([P, CH], F32, tag="r32")
            nc.vector.reciprocal_approx_fast(r32[:], ps[:])
            # Sqrt ACTs all precede the first Exp -> one table swap total
            nc.scalar.activation(rs_i[:, sl], r32[:], AF.Sqrt)
            for k in range(kt):
                nc.vector.tensor_tensor(img8[:, rc, k, :],
                                        imgT_sb[:, rc, k, :],
                                        rs_i[:, sl], ALU.mult)

        # --- local text norms for the first N_LOCAL chunks ------------------
        # (cover the norms-AllGather latency; squares for c0/c1 on GpSimd,
        # the rest on DVE after the image quantize)
        def prep_local(c, eng):
            sq = sq8p.tile([P, kt, CH], FP8, tag="sq8")
            eng.tensor_tensor(sq[:], txt8s[:, c, :, :], txt8s[:, c, :, :],
                              ALU.mult)
            ps = psum_n.tile([P, CH], F32, tag="nps", bufs=2)
            for t in range(kp):
                nc.tensor.matmul(ps[:], ones8_sb[:],
                                 sq[:, 2 * t:2 * t + 2, :],
                                 start=(t == 0), stop=(t == kp - 1),
                                 perf_mode=DR)
            row = rsp.tile([1, CH], F32, tag="row", bufs=8)
            nc.vector.tensor_copy(row[:], ps[0:1, :])
            # partition-transpose [1, 512] -> [128, 4] via a DRAM bounce
            nc.sync.dma_start(bnc[0:1, c, :], row[:])
            aT = rsp.tile([P, jb_n], F32, tag="aT", bufs=8)
            nc.sync.dma_start(
                aT[:], bnc[0:1, c, :].rearrange("a (x p) -> (a p) x", p=P))
            t1 = rsp.tile([P, jb_n], F32, tag="t1l", bufs=8)
            _newton_rsqrt(nc, scl_loc[:, c, :], aT[:], t1[:], scale=inv_t)

        prep_local(0, nc.gpsimd)
        prep_local(1, nc.gpsimd)

        # --- own-slab text ssq -> AllGather -> all 8192 text norms ----------
        for rc in range(rc_ch):
            ps2 = psum_n.tile([P, CH], F32, tag="nps", bufs=2)
            col_ssq_bf16(totT_sb[:, rc, :, :], ps2)
            nc.vector.tensor_copy(tssq[:, rc, :], ps2[:])
        # p-major transpose into DRAM: agin[(p x)] <- tssq rows [(x p)]
        arow = v1.tile([1, rows], F32, tag="arow")
        nc.vector.tensor_copy(arow[0:1, 0:CH], tssq[0:1, 0, :])
        nc.vector.tensor_copy(arow[0:1, CH:rows], tssq[0:1, 1, :])
        nc.sync.dma_start(
            agin[0:1, :].rearrange("a (p x) -> (a x) p", p=P),
            arow[0:1, :])
        nc.gpsimd.collective_compute(
            "AllGather", ALU.bypass, replica_groups=grp,
            ins=[agin[:].opt()], outs=[agout[:].opt()])
        # gather slots are rank-ordered == natural global column order
        gat = v1.tile([P, n_ch * jb_n], F32, tag="gat")
        nc.sync.dma_start(
            gat[:].rearrange("p (m x) -> p m x", x=rows // P),
            agout[0:1, :].rearrange("a (m p x) -> (a p) m x",
                                    p=P, x=rows // P))
        gt1 = v1.tile([P, n_ch * jb_n], F32, tag="gt1")
        _newton_rsqrt(nc, scl_gat[:], gat[:], gt1[:], scale=inv_t)

        # remaining local-chunk norms on DVE (after quantize in issue order)
        for c in range(2, N_LOCAL):
            prep_local(c, nc.vector)

        # --- diagonal dots (for Draw) --------------------------------------
        for rc in range(rc_ch):
            sl = slice(rc * CH, (rc + 1) * CH)
            prod = sqbp.tile([P, kt, CH], BF16, tag="sqb")
            nc.vector.tensor_tensor(prod[:], imgT_sb[:, rc, :, :],
                                    totT_sb[:, rc, :, :], ALU.mult)
            dps = psum_n.tile([P, CH], F32, tag="nps", bufs=2)
            for k in range(kt):
                nc.tensor.matmul(dps[:], onesb_sb[:], prod[:, k, :],
                                 start=(k == 0), stop=(k == kt - 1))
            nc.vector.tensor_copy(ddv[:, sl], dps[:])
        # rs_to = rsqrt(own text ssq), [P, rows] replicated
        for rc in range(rc_ch):
            sl = slice(rc * CH, (rc + 1) * CH)
            tt1 = rsp.tile([P, CH], F32, tag="tt1")
            _newton_rsqrt(nc, rs_to[:, sl], tssq[:, rc, :], tt1[:])
        nc.vector.tensor_tensor(ddv[:], ddv[:], rs_i[:], ALU.mult)
        nc.vector.tensor_tensor(ddv[:], ddv[:], rs_to[:], ALU.mult)
        nc.vector.tensor_reduce(pay[:, 65:66], ddv[:],
                                axis=mybir.AxisListType.X, op=ALU.add)

        # --- main loop: per text chunk --------------------------------------
        for c in range(n_ch):
            exs = exsp.tile([P, jb_n, rows], FP8, tag="exs")
            for jb in range(jb_n):
                col = c * jb_n + jb
                mm = psum_mm.tile([P, rows], F32, tag="mm")
                for rc in range(rc_ch):
                    for t in range(kp):
                        nc.tensor.matmul(
                            mm[:, rc * CH:(rc + 1) * CH],
                            txt8s[:, c, 2 * t:2 * t + 2, jb * P:(jb + 1) * P],
                            img8[:, rc, 2 * t:2 * t + 2, :],
                            start=(t == 0), stop=(t == kp - 1), perf_mode=DR)
                scl = (scl_loc[:, c, jb:jb + 1] if c < N_LOCAL
                       else scl_gat[:, col:col + 1])
                nc.scalar.activation(
                    exs[:, jb, :], mm[:], AF.Exp, bias=ebias[:, 0:1],
                    scale=scl, accum_out=pay[:, col:col + 1])
            # rowsum partials: DoubleRow ones-matmuls into persistent PSUM
            for u in range(jb_n // 2):
                for h in range(rc_ch):
                    nc.tensor.matmul(
                        rps[:, h * CH:(h + 1) * CH], ones8_sb[:],
                        exs[:, 2 * u:2 * u + 2, h * CH:(h + 1) * CH],
                        start=(c == 0 and u == 0),
                        stop=(c == n_ch - 1 and u == jb_n // 2 - 1),
                        perf_mode=DR)

        # --- local scalars ---------------------------------------------------
        lnr = v1.tile([P, rows], F32, tag="lnr")
        nc.scalar.activation(lnr[:], rps[:], AF.Ln)   # same table as Exp
        nc.vector.tensor_reduce(pay[:, 64:65], lnr[:],
                                axis=mybir.AxisListType.X, op=ALU.add)

        # --- tail AllGather: [128, 68] bf16 payload, p-major ---------------
        # cols 0-63: cparts as bf16; cols 64-67: R/Draw f32 bitcast pairs
        nc.vector.tensor_copy(pay_bf[:, 0:64], pay[:, 0:64])
        nc.vector.tensor_copy(pay_bf[:, 64:68].bitcast(F32), pay[:, 64:66])
        nc.sync.dma_start(
            cbuf[0:1, :].rearrange("a (p x) -> (a p) x", p=P), pay_bf[:])
        nc.gpsimd.collective_compute(
            "AllGather", ALU.bypass, replica_groups=grp,
            ins=[cbuf[:].opt()], outs=[cbuf_out[:].opt()])
        nc.sync.dma_start(
            recv[:],
            cbuf_out[0:1, :].rearrange("a (m p x) -> (a p) m x",
                                       p=P, x=WP))
        # sum colsums in f32; sum the bitcast scalar pairs as f32
        acc = v1.tile([P, 64], F32, tag="acc")
        accs = v1.tile([P, 2], F32, tag="accs")
        nc.vector.tensor_tensor(acc[:], recv[:, 0, 0:64], recv[:, 1, 0:64],
                                ALU.add)
        nc.vector.tensor_tensor(accs[:], recv[:, 0, 64:68].bitcast(F32),
                                recv[:, 1, 64:68].bitcast(F32), ALU.add)
        for m in range(2, n_cores):
            nc.vector.tensor_tensor(acc[:], acc[:], recv[:, m, 0:64], ALU.add)
            nc.vector.tensor_tensor(accs[:], accs[:],
                                    recv[:, m, 64:68].bitcast(F32), ALU.add)
        ln_cs = v1.tile([P, 64], F32, tag="ln_cs")
        nc.scalar.activation(ln_cs[:], acc[:], AF.Ln)
        nc.vector.tensor_reduce(vecs[:, 3:4], ln_cs[:],
                                axis=mybir.AxisListType.X, op=ALU.add)
        nc.gpsimd.partition_all_reduce(vecs[:, 4:5], vecs[:, 3:4], channels=P,
                                       reduce_op=bass_isa.ReduceOp.add)

        # loss = (C - LOG_OFF) + (R + L - 2C*Draw) / (2N)
        fin = v1.tile([P, 8], F32, tag="fin")
        nc.vector.tensor_tensor(fin[0:1, 0:1], accs[0:1, 0:1], vecs[0:1, 4:5],
                                ALU.add)                        # R + L
        nc.vector.tensor_scalar_mul(fin[0:1, 1:2], accs[0:1, 1:2],
                                    float(-2.0 * inv_t))        # -2C*Draw
        nc.vector.tensor_tensor(fin[0:1, 2:3], fin[0:1, 0:1], fin[0:1, 1:2],
                                ALU.add)
        nc.scalar.activation(fin[0:1, 3:4], fin[0:1, 2:3], AF.Copy,
                             bias=float(inv_t - log_off),
                             scale=float(1.0 / (2 * n)))
        nc.sync.dma_start(out[0:1, 0:1], fin[0:1, 3:4])


def _permute(xT, ch):
    """[d, cols] -> [cols//ch, 128, d//128, ch] (SBUF tile layout, dense)."""
    d, cols = xT.shape
    return np.ascontiguousarray(
        xT.reshape(d // 128, 128, cols // ch, ch).transpose(2, 1, 0, 3))


def make_in_maps(image_features, text_features, n=N, d=D, n_cores=N_CORES):
    image_features = np.asarray(image_features, dtype=np.float32)
    text_features = np.asarray(text_features, dtype=np.float32)
    rows = n // n_cores
    txt8 = _permute(text_features.T.astype(ml_dtypes.float8_e4m3), 512)
    ones8 = np.ones((128, 256), dtype=ml_dtypes.float8_e4m3)
    onesb = np.ones((128, 128), dtype=ml_dtypes.bfloat16)
    maps = []
    for m in range(n_cores):
        sl = slice(m * rows, (m + 1) * rows)
        maps.append({
            "imgT": _permute(
                image_features[sl].T.astype(ml_dtypes.bfloat16), 512),
            "totT": _permute(
                text_features[sl].T.astype(ml_dtypes.bfloat16), 512),
            "txt8": txt8,
            "ones8": ones8,
            "onesb": onesb,
        })
    return maps


_CACHE = {}
_LOCK = threading.Lock()


def _get_nc():
    with _LOCK:
        if "nc" not in _CACHE:
            _CACHE["nc"] = build_nc()
        return _CACHE["nc"]


def kernel(image_features, text_features):
    image_features = np.asarray(image_features, dtype=np.float32)
    text_features = np.asarray(text_features, dtype=np.float32)
    assert image_features.shape == (N, D) and text_features.shape == (N, D)
    nc = _get_nc()
    in_maps = make_in_maps(image_features, text_features)
    res = run_bass_kernel_spmd(nc, in_maps, list(range(N_CORES)))
    val = np.float32(res.results[0]["out"][0, 0])
    return np.array(val, dtype=np.float32)
